# revision 1
# baseline (speedup 1.0000x reference)
"""Trainium2 Bass kernel for nn_HamiltonianVersorNN.

Math: the reference energy reads only blade-0 of the final layer, and the
versor gate h*sigmoid(h[...,0:1]) makes blade-0 evolve as elementwise SiLU.
Backprop therefore collapses exactly to a 2-layer SiLU MLP on blade-0:

    a1 = A x + c1            A  = W1 @ W_in[:, ::32].T          [32, 6]
    a2 = W2 silu(a1) + c2    c1 = W1 @ b_in[::32] + b1[:, 0]
    dx = A.T (W2.T (w3 * silu'(a2)) * silu'(a1))
    out = x + dt * [dx[3:6], -dx[0:3]]

(verified to rel err 4.6e-8 against the jax reference).

Sharding: pure data parallel over B*S*N positions, 8 cores, 16384
positions/core. On-chip layout packs 4 tokens per 128-partition column
(partition 32*tl + c holds channel c of token 4g+tl) so the W2 matmuls
contract over the full 128 partitions via block-diagonal stationaries.

silu'(x) uses ActivationFunctionType.Derivative_silu; silu(a1) is built
from Tanh (same ACT table set as Derivative_silu -> no table switches):
silu(z) = z * (1 + tanh(z/2)) / 2.
"""

import sys

import numpy as np

if "/opt/trn_rl_repo" not in sys.path:
    sys.path.insert(0, "/opt/trn_rl_repo")

import concourse.bass as bass
import concourse.tile as tile
from concourse import mybir

AF = mybir.ActivationFunctionType
F32 = mybir.dt.float32

N_CORES = 8
B, S, N, D = 32, 256, 16, 6
HIDDEN = 32
BLADES = 32
DT = 0.01

TOK_TOTAL = B * S * N          # 131072 positions
TOK_CORE = TOK_TOTAL // N_CORES  # 16384
TPC = 4                        # tokens packed per 128-partition column
GROUPS = TOK_CORE // TPC       # 4096 columns per core
FD = 512                       # free-dim per tile (1 PSUM bank fp32)
N_TILES = GROUPS // FD         # 8

KP = TPC * D                   # 24 partitions for x / out


def _build_nc():
    nc = bass.Bass()

    xg = nc.dram_tensor("xg", [KP, GROUPS], F32, kind="ExternalInput")
    l1 = nc.dram_tensor("l1", [KP, 128], F32, kind="ExternalInput")
    l2 = nc.dram_tensor("l2", [128, 128], F32, kind="ExternalInput")
    l3 = nc.dram_tensor("l3", [128, 128], F32, kind="ExternalInput")
    l4 = nc.dram_tensor("l4", [128, KP], F32, kind="ExternalInput")
    eye = nc.dram_tensor("eye", [KP, KP], F32, kind="ExternalInput")
    c1r = nc.dram_tensor("c1r", [1, 128], F32, kind="ExternalInput")
    c2r = nc.dram_tensor("c2r", [1, 128], F32, kind="ExternalInput")
    outg = nc.dram_tensor("outg", [KP, GROUPS], F32, kind="ExternalOutput")

    with tile.TileContext(nc) as tc:
        with (
            tc.tile_pool(name="consts", bufs=1) as consts,
            tc.tile_pool(name="xin", bufs=4) as xin,
            tc.tile_pool(name="work", bufs=3) as work,
            tc.tile_pool(name="ps", bufs=2, space="PSUM") as ps,
        ):
            sb_l1 = consts.tile([KP, 128], F32)
            nc.sync.dma_start(out=sb_l1[:], in_=l1[:])
            sb_l2 = consts.tile([128, 128], F32)
            nc.sync.dma_start(out=sb_l2[:], in_=l2[:])
            sb_l3 = consts.tile([128, 128], F32)
            nc.sync.dma_start(out=sb_l3[:], in_=l3[:])
            sb_l4 = consts.tile([128, KP], F32)
            nc.sync.dma_start(out=sb_l4[:], in_=l4[:])
            sb_eye = consts.tile([KP, KP], F32)
            nc.sync.dma_start(out=sb_eye[:], in_=eye[:])
            sb_c1r = consts.tile([1, 128], F32)
            nc.sync.dma_start(out=sb_c1r[:], in_=c1r[:])
            sb_c2r = consts.tile([1, 128], F32)
            nc.sync.dma_start(out=sb_c2r[:], in_=c2r[:])
            sb_ones = consts.tile([1, FD], F32)
            nc.vector.memset(sb_ones[:], 1.0)

            # Dummy first activation: walrus attaches the ACT table load to
            # the first Activation instruction, which can then carry only a
            # single sync wait. Give it a single-wait warm-up op.
            warm = consts.tile([1, 128], F32)
            nc.scalar.activation(warm[:], sb_c2r[:], AF.Derivative_silu)

            for t in range(N_TILES):
                cs = bass.ts(t, FD)

                sb_x = xin.tile([KP, FD], F32, tag="x")
                nc.sync.dma_start(out=sb_x[:], in_=xg[:, cs])

                # a1 = blockdiag(A) @ x + c1 (bias via rank-1 accumulate)
                a1 = ps.tile([128, FD], F32, tag="a1")
                nc.tensor.matmul(a1[:], sb_l1[:], sb_x[:], start=True, stop=False)
                nc.tensor.matmul(a1[:], sb_c1r[:], sb_ones[:], start=False, stop=True)

                # d1 = silu'(a1)
                d1 = work.tile([128, FD], F32, tag="d1")
                nc.scalar.activation(d1[:], a1[:], AF.Derivative_silu)
                # tau = tanh(0.5*a1)
                tau = work.tile([128, FD], F32, tag="tau")
                nc.scalar.activation(tau[:], a1[:], AF.Tanh, scale=0.5)
                # p1 = 0.5*tau + 0.5 = sigmoid(a1)
                p1 = work.tile([128, FD], F32, tag="p1")
                nc.vector.tensor_scalar(
                    p1[:], tau[:], 0.5, 0.5, mybir.AluOpType.mult, mybir.AluOpType.add
                )
                # h1 = silu(a1) = a1 * sigmoid(a1)
                h1 = work.tile([128, FD], F32, tag="h1")
                nc.vector.tensor_mul(h1[:], a1[:], p1[:])

                # a2 = blockdiag(W2) @ h1 + c2
                a2 = ps.tile([128, FD], F32, tag="a2")
                nc.tensor.matmul(a2[:], sb_l2[:], h1[:], start=True, stop=False)
                nc.tensor.matmul(a2[:], sb_c2r[:], sb_ones[:], start=False, stop=True)

                # d2 = silu'(a2)
                d2 = work.tile([128, FD], F32, tag="d2")
                nc.scalar.activation(d2[:], a2[:], AF.Derivative_silu)

                # v1 = blockdiag(diag(w3) W2)^T-contraction @ d2
                v1 = ps.tile([128, FD], F32, tag="v1")
                nc.tensor.matmul(v1[:], sb_l3[:], d2[:], start=True, stop=True)

                # g1 = v1 * d1
                g1 = work.tile([128, FD], F32, tag="g1")
                nc.vector.tensor_mul(g1[:], v1[:], d1[:])

                # dxJ = blockdiag(Bout) @ g1 (symplectic swap + dt folded in)
                po = ps.tile([128, FD], F32, tag="po")
                nc.tensor.matmul(po[:KP, :], sb_l4[:], g1[:], start=True, stop=True)

                # out = x + dxJ  (PSUM + SBUF -> SBUF, then DMA out)
                sb_o = work.tile([KP, FD], F32, tag="o")
                nc.vector.tensor_add(sb_o[:], po[:KP, :], sb_x[:])
                nc.sync.dma_start(out=outg[:, cs], in_=sb_o[:])

    return nc


def _split_multi_waits(nc):
    """This walrus build rejects engine instructions carrying more than one
    sync wait ("Too many sync wait commands"). Hoist all but one wait of
    each instruction onto standalone NoOps issued just before it on the
    same engine (engines execute their queue in order, so semantics are
    preserved)."""
    for f in nc.m.functions:
        for b in f.blocks:
            insts = list(b.instructions)
            out = []
            changed = False
            for inst in insts:
                # This walrus build also rejects the raw-ISA
                # EVENT_SEMAPHORE_RANGE_CLEAR Tile emits at context end
                # ("ISA wrong length" — ISA table version skew). The NEFF
                # preamble re-initializes semaphores, so drop it.
                if (
                    type(inst).__name__ == "InstISA"
                    and getattr(inst, "op_name", "") == "EVENT_SEMAPHORE_RANGE_CLEAR"
                ):
                    changed = True
                    continue
                si = getattr(inst, "sync_info", None)
                waits = list(si.on_wait) if si is not None and si.on_wait else []
                if len(waits) > 1:
                    changed = True
                    for k, w in enumerate(waits[:-1]):
                        nop = mybir.InstNoOp(name=f"{inst.name}-w{k}", ins=[], outs=[])
                        nop.engine = inst.engine
                        nop.sync_info = mybir.SyncInfo(on_wait=[w], on_update=[])
                        out.append(nop)
                    inst.sync_info = mybir.SyncInfo(
                        on_wait=[waits[-1]], on_update=list(si.on_update or [])
                    )
                out.append(inst)
            if changed:
                b.instructions = out
    return nc


_NC_CACHE = None


def _get_nc():
    global _NC_CACHE
    if _NC_CACHE is None:
        _NC_CACHE = _split_multi_waits(_build_nc())
    return _NC_CACHE


def _prep_weights(W_in, b_in, W1, b1, W2, b2, W3, b3):
    """Host-side constant folding into the kernel's stationary layouts."""
    W_in = np.asarray(W_in, np.float64)
    b_in = np.asarray(b_in, np.float64)
    W1 = np.asarray(W1, np.float64)
    b1 = np.asarray(b1, np.float64)
    W2 = np.asarray(W2, np.float64)
    b2 = np.asarray(b2, np.float64)
    W3 = np.asarray(W3, np.float64)

    Win0 = W_in[:, ::BLADES]            # [6, 8]
    bin0 = b_in[::BLADES]               # [8]
    A = W1 @ Win0.T                     # [32, 6]
    c1 = W1 @ bin0 + b1[:, 0]           # [32]
    c2 = b2[:, 0]                       # [32]
    w3 = W3[0, :]                       # [32]

    # Bout[d, c]: out[d] += dt*dx[d+3] (d<3), -dt*dx[d-3] (d>=3); dx = A^T g1
    Bout = np.zeros((D, HIDDEN))
    Bout[0:3, :] = DT * A[:, 3:6].T
    Bout[3:6, :] = -DT * A[:, 0:3].T

    l1 = np.zeros((KP, 128), np.float32)
    l2 = np.zeros((128, 128), np.float32)
    l3 = np.zeros((128, 128), np.float32)
    l4 = np.zeros((128, KP), np.float32)
    for tl in range(TPC):
        # l1[6tl+d, 32tl+c] = A[c, d]
        l1[6 * tl : 6 * tl + 6, 32 * tl : 32 * tl + 32] = A.T.astype(np.float32)
        # l2[32tl+ci, 32tl+co] = W2[co, ci]
        l2[32 * tl : 32 * tl + 32, 32 * tl : 32 * tl + 32] = W2.T.astype(np.float32)
        # l3[32tl+co, 32tl+ci] = w3[co] * W2[co, ci]
        l3[32 * tl : 32 * tl + 32, 32 * tl : 32 * tl + 32] = (
            w3[:, None] * W2
        ).astype(np.float32)
        # l4[32tl+c, 6tl+d] = Bout[d, c]
        l4[32 * tl : 32 * tl + 32, 6 * tl : 6 * tl + 6] = Bout.T.astype(np.float32)

    eye = np.eye(KP, dtype=np.float32)

    c1row = np.zeros((1, 128), np.float32)
    c2row = np.zeros((1, 128), np.float32)
    for tl in range(TPC):
        c1row[0, 32 * tl : 32 * tl + 32] = c1.astype(np.float32)
        c2row[0, 32 * tl : 32 * tl + 32] = c2.astype(np.float32)

    return {
        "l1": l1,
        "l2": l2,
        "l3": l3,
        "l4": l4,
        "eye": eye,
        "c1r": c1row,
        "c2r": c2row,
    }


def _shard_x(x):
    """[B,S,N,D] -> list of per-core [24, GROUPS] arrays."""
    xf = np.ascontiguousarray(np.asarray(x, np.float32)).reshape(TOK_TOTAL, D)
    shards = []
    for c in range(N_CORES):
        xc = xf[c * TOK_CORE : (c + 1) * TOK_CORE]          # [16384, 6]
        xgc = np.ascontiguousarray(
            xc.reshape(GROUPS, TPC, D).transpose(1, 2, 0).reshape(KP, GROUPS)
        )
        shards.append(xgc)
    return shards


def _unshard_out(outs):
    """list of per-core [24, GROUPS] -> [B,S,N,D]."""
    full = np.empty((TOK_TOTAL, D), np.float32)
    for c, og in enumerate(outs):
        oc = (
            np.asarray(og)
            .reshape(TPC, D, GROUPS)
            .transpose(2, 0, 1)
            .reshape(TOK_CORE, D)
        )
        full[c * TOK_CORE : (c + 1) * TOK_CORE] = oc
    return full.reshape(B, S, N, D)


# Test-harness knobs (ignored in normal use): set kernel._TRACE = True to
# collect an NTFF profile; the BassKernelResults lands in kernel._LAST_RES.
_TRACE = False
_LAST_RES = None


def kernel(x, W_in, b_in, W1, b1, W2, b2, W3, b3):
    global _LAST_RES
    from concourse.bass_utils import run_bass_kernel_spmd

    nc = _get_nc()
    consts = _prep_weights(W_in, b_in, W1, b1, W2, b2, W3, b3)
    shards = _shard_x(x)
    in_maps = [{"xg": shards[c], **consts} for c in range(N_CORES)]
    res = run_bass_kernel_spmd(nc, in_maps, list(range(N_CORES)), trace=_TRACE)
    _LAST_RES = res
    return _unshard_out([res.results[c]["outg"] for c in range(N_CORES)])



# revision 7
# speedup vs baseline: 1.6273x; 1.6273x over previous
"""Trainium2 Bass kernel for nn_HamiltonianVersorNN.

Math: the reference energy reads only blade-0 of the final layer, and the
versor gate h*sigmoid(h[...,0:1]) makes blade-0 evolve as elementwise SiLU.
Backprop therefore collapses exactly to a 2-layer SiLU MLP on blade-0:

    z1 = A x + c1            A  = W1 @ W_in[:, ::32].T          [32, 6]
    z2 = W2 silu(z1) + c2    c1 = W1 @ b_in[::32] + b1[:, 0]
    dx = A.T (W2.T (w3 * silu'(z2)) * silu'(z1))
    out = x + dt * [dx[3:6], -dx[0:3]]

Performance structure (vs the fp32 block-diag baseline):
  * float32r matmuls (FP22 multiply, 1 cycle/row at N=512, vs fp32's 4).
  * silu(z1) never materialized: with u1 = z1*tanh(z1/2),
    silu(z1) = (z1 + u1)/2, so W2 silu(z1) = 0.5*(W2 A)x + 0.5*W2 u1 +
    0.5*W2 c1 folds into two stationaries. ACT does only tanh + 2x
    silu' per tile; DVE does 2 muls.
  * biases ride a host-provided ones-row on the x tile (row 24), so no
    rank-1 bias matmuls and no ACT bias chains.
  * the last matmul runs bf16 so its PSUM output can col-tile: chunk
    parity p lands at partitions 32p+{0..23} of one [56, 512] bank,
    halving PSUM readout cost; the x-passthrough is added during the
    readout against a second, fold-layout copy of x (xb).

Sharding: pure data parallel over B*S*N positions, 8 cores, 16384
positions/core; partition 32*tl + c holds channel c of token 4g+tl.
(fp22 numerics verified on CPU: rel err 4.7e-5 vs jax reference.)
"""

import sys

import numpy as np

if "/opt/trn_rl_repo" not in sys.path:
    sys.path.insert(0, "/opt/trn_rl_repo")

import concourse.bass as bass
import concourse.tile as tile
from concourse import mybir

AF = mybir.ActivationFunctionType
F32 = mybir.dt.float32
F32R = mybir.dt.float32r

N_CORES = 8
B, S, N, D = 32, 256, 16, 6
HIDDEN = 32
BLADES = 32
DT = 0.01

TOK_TOTAL = B * S * N            # 131072 positions
TOK_CORE = TOK_TOTAL // N_CORES  # 16384
TPC = 4                          # tokens packed per 128-partition column
GROUPS = TOK_CORE // TPC         # 4096 columns per core
FD = 512                         # free-dim per tile (1 PSUM bank fp32)
N_TILES = GROUPS // FD           # 8

KP = TPC * D                     # 24 x/out partitions per lane block
XROWS = KP + 1                   # + ones row for bias folding
ORROWS = 56                      # folded out rows: 32p + (0..23), p in {0,1}
SFD = 2 * FD                     # 1024-column super-tile (ACT/DVE op span)
BF16 = mybir.dt.bfloat16
OGCOLS = GROUPS // 2             # 2048 output columns (2 chunks/bank)


def _build_nc():
    nc = bass.Bass()

    xg = nc.dram_tensor("xg", [XROWS, GROUPS], F32R, kind="ExternalInput")
    l1 = nc.dram_tensor("l1", [XROWS, 128], F32R, kind="ExternalInput")
    l2a = nc.dram_tensor("l2a", [XROWS, 128], F32R, kind="ExternalInput")
    l2b = nc.dram_tensor("l2b", [128, 128], F32R, kind="ExternalInput")
    l3 = nc.dram_tensor("l3", [128, 128], F32R, kind="ExternalInput")
    l4 = nc.dram_tensor("l4", [128, KP], BF16, kind="ExternalInput")
    xb = nc.dram_tensor("xb", [ORROWS, OGCOLS], F32, kind="ExternalInput")
    og = nc.dram_tensor("og", [2 * KP, OGCOLS], F32, kind="ExternalOutput")

    with tile.TileContext(nc) as tc:
        with (
            tc.tile_pool(name="consts", bufs=1) as consts,
            tc.tile_pool(name="xin", bufs=1) as xin,
            tc.tile_pool(name="work", bufs=2) as work,
            tc.tile_pool(name="outp", bufs=2) as outp,
            tc.tile_pool(name="ps", bufs=3, space="PSUM") as ps,
            tc.tile_pool(name="pso", bufs=2, space="PSUM") as pso,
        ):
            sb_l1 = consts.tile([XROWS, 128], F32R)
            nc.sync.dma_start(out=sb_l1[:], in_=l1[:])
            sb_l2a = consts.tile([XROWS, 128], F32R)
            nc.sync.dma_start(out=sb_l2a[:], in_=l2a[:])
            sb_l2b = consts.tile([128, 128], F32R)
            nc.sync.dma_start(out=sb_l2b[:], in_=l2b[:])
            sb_l3 = consts.tile([128, 128], F32R)
            nc.sync.dma_start(out=sb_l3[:], in_=l3[:])
            sb_l4 = consts.tile([128, KP], BF16)
            nc.sync.dma_start(out=sb_l4[:], in_=l4[:])
            sb_xb = consts.tile([ORROWS, OGCOLS], F32)
            nc.sync.dma_start(out=sb_xb[:], in_=xb[:])

            # x input: one SBUF tile per 1024-column group, loaded early so
            # compute on group g overlaps the DMA of group g+1.
            xw = []
            for g in range(N_TILES // 2):
                xt = xin.tile([XROWS, 2 * FD], F32R, name=f"xw{g}")
                nc.sync.dma_start(out=xt[:], in_=xg[:, bass.ts(g, 2 * FD)])
                xw.append(xt)

            # Dummy first activation: walrus attaches the ACT table load to
            # the first Activation instruction, which can then carry only a
            # single sync wait. Give it a single-wait warm-up op.
            warm_in = consts.tile([1, 128], F32)
            nc.vector.memset(warm_in[:], 0.25)
            warm = consts.tile([1, 128], F32)
            nc.scalar.activation(warm[:], warm_in[:], AF.Derivative_silu)

            for s in range(N_TILES // 2):
                xsl = [xw[s][:, bass.ts(p, FD)] for p in range(2)]

                # z1 = blockdiag(A) @ x + c1 (bias via ones row)
                z1 = ps.tile([128, SFD], F32, tag="z")
                for p in range(2):
                    nc.tensor.matmul(
                        z1[:, bass.ts(p, FD)], sb_l1[:], xsl[p], start=True, stop=True
                    )

                # t1 = tanh(z1/2), d1 = silu'(z1)
                t1 = work.tile([128, SFD], F32, tag="t1")
                nc.scalar.activation(t1[:], z1[:], AF.Tanh, scale=0.5)
                d1 = work.tile([128, SFD], F32, tag="d1")
                nc.scalar.activation(d1[:], z1[:], AF.Derivative_silu)

                # u1 = z1 * t1   (silu(z1) = (z1 + u1)/2, kept implicit)
                u1 = work.tile([128, SFD], F32R, tag="u1")
                nc.vector.tensor_mul(u1[:], z1[:], t1[:])

                # z2 = 0.5*W2A x + (c2 + 0.5*W2 c1) + 0.5*W2 u1
                z2 = ps.tile([128, SFD], F32, tag="z")
                for p in range(2):
                    zs = z2[:, bass.ts(p, FD)]
                    nc.tensor.matmul(zs, sb_l2a[:], xsl[p], start=True, stop=False)
                    nc.tensor.matmul(
                        zs, sb_l2b[:], u1[:, bass.ts(p, FD)], start=False, stop=True
                    )

                # d2 = silu'(z2)
                d2 = work.tile([128, SFD], F32R, tag="d2")
                nc.scalar.activation(d2[:], z2[:], AF.Derivative_silu)

                # v1 = blockdiag(w3*W2)^T-contraction @ d2
                v1 = ps.tile([128, SFD], F32, tag="z")
                for p in range(2):
                    nc.tensor.matmul(
                        v1[:, bass.ts(p, FD)],
                        sb_l3[:],
                        d2[:, bass.ts(p, FD)],
                        start=True,
                        stop=True,
                    )

                # g1 = v1 * d1 (bf16: feeds the col-tiled bf16 mm4)
                g1 = work.tile([128, SFD], BF16, tag="g1")
                nc.vector.tensor_mul(g1[:], v1[:], d1[:])

                # po[32p + (0..23)] = dt*J*(A^T g1), parity-folded into 1 bank
                po = pso.tile([ORROWS, FD], F32, tag="po")
                for p in range(2):
                    nc.tensor.matmul(
                        po[32 * p : 32 * p + KP, :],
                        sb_l4[:],
                        g1[:, bass.ts(p, FD)],
                        start=True,
                        stop=True,
                    )

                # out = x + po, added during PSUM readout against fold-layout x
                osb = outp.tile([ORROWS, FD], F32, tag="osb")
                nc.vector.tensor_add(osb[:], po[:], sb_xb[:, bass.ts(s, FD)])
                cs = bass.ts(s, FD)
                nc.sync.dma_start(out=og[0:KP, cs], in_=osb[0:KP, :])
                nc.sync.dma_start(out=og[KP : 2 * KP, cs], in_=osb[32 : 32 + KP, :])

    return nc


def _split_multi_waits(nc):
    """This walrus build rejects engine instructions carrying more than one
    sync wait ("Too many sync wait commands"). Hoist all but one wait of
    each instruction onto standalone NoOps issued just before it on the
    same engine (engines execute their queue in order, so semantics are
    preserved)."""
    for f in nc.m.functions:
        for b in f.blocks:
            insts = list(b.instructions)
            out = []
            changed = False
            for inst in insts:
                # This walrus build also rejects the raw-ISA
                # EVENT_SEMAPHORE_RANGE_CLEAR Tile emits at context end
                # ("ISA wrong length" — ISA table version skew). The NEFF
                # preamble re-initializes semaphores, so drop it.
                if (
                    type(inst).__name__ == "InstISA"
                    and getattr(inst, "op_name", "") == "EVENT_SEMAPHORE_RANGE_CLEAR"
                ):
                    changed = True
                    continue
                si = getattr(inst, "sync_info", None)
                waits = list(si.on_wait) if si is not None and si.on_wait else []
                if len(waits) > 1:
                    changed = True
                    for k, w in enumerate(waits[:-1]):
                        nop = mybir.InstNoOp(name=f"{inst.name}-w{k}", ins=[], outs=[])
                        nop.engine = inst.engine
                        nop.sync_info = mybir.SyncInfo(on_wait=[w], on_update=[])
                        out.append(nop)
                    inst.sync_info = mybir.SyncInfo(
                        on_wait=[waits[-1]], on_update=list(si.on_update or [])
                    )
                out.append(inst)
            if changed:
                b.instructions = out
    return nc


_NC_CACHE = None


def _get_nc():
    global _NC_CACHE
    if _NC_CACHE is None:
        _NC_CACHE = _split_multi_waits(_build_nc())
    return _NC_CACHE


def _prep_weights(W_in, b_in, W1, b1, W2, b2, W3, b3):
    """Host-side constant folding into the kernel's stationary layouts."""
    W_in = np.asarray(W_in, np.float64)
    b_in = np.asarray(b_in, np.float64)
    W1 = np.asarray(W1, np.float64)
    b1 = np.asarray(b1, np.float64)
    W2 = np.asarray(W2, np.float64)
    b2 = np.asarray(b2, np.float64)
    W3 = np.asarray(W3, np.float64)

    Win0 = W_in[:, ::BLADES]            # [6, 8]
    bin0 = b_in[::BLADES]               # [8]
    A = W1 @ Win0.T                     # [32, 6]
    c1 = W1 @ bin0 + b1[:, 0]           # [32]
    c2 = b2[:, 0]                       # [32]
    w3 = W3[0, :]                       # [32]

    W2A = 0.5 * (W2 @ A)                # [32, 6]
    cz2 = 0.5 * (W2 @ c1) + c2          # [32]

    # Bout[d, c]: out[d] += dt*dx[d+3] (d<3), -dt*dx[d-3] (d>=3); dx = A^T g1
    Bout = np.zeros((D, HIDDEN))
    Bout[0:3, :] = DT * A[:, 3:6].T
    Bout[3:6, :] = -DT * A[:, 0:3].T

    import ml_dtypes

    l1 = np.zeros((XROWS, 128), np.float32)
    l2a = np.zeros((XROWS, 128), np.float32)
    l2b = np.zeros((128, 128), np.float32)
    l3 = np.zeros((128, 128), np.float32)
    l4 = np.zeros((128, KP), ml_dtypes.bfloat16)
    for tl in range(TPC):
        r0, c0 = 6 * tl, 32 * tl
        # z1[32tl+c] = sum_d A[c,d] x[d] + c1[c]
        l1[r0 : r0 + 6, c0 : c0 + 32] = A.T.astype(np.float32)
        l1[KP, c0 : c0 + 32] = c1.astype(np.float32)
        # z2 partial from x: 0.5*(W2A)[j,d] + bias row
        l2a[r0 : r0 + 6, c0 : c0 + 32] = W2A.T.astype(np.float32)
        l2a[KP, c0 : c0 + 32] = cz2.astype(np.float32)
        # z2 partial from u1: 0.5*W2[j,i]
        l2b[c0 : c0 + 32, c0 : c0 + 32] = (0.5 * W2.T).astype(np.float32)
        # v1[j] = sum_i w3[i] W2[i,j] d2[i]
        l3[c0 : c0 + 32, c0 : c0 + 32] = (w3[:, None] * W2).astype(np.float32)
        # out[6tl+d] += Bout[d, c] g1[32tl+c]
        l4[c0 : c0 + 32, r0 : r0 + 6] = Bout.T.astype(ml_dtypes.bfloat16)

    return {"l1": l1, "l2a": l2a, "l2b": l2b, "l3": l3, "l4": l4}


def _shard_x(x):
    """[B,S,N,D] -> per-core ([25, GROUPS] lane layout + ones row,
    [56, 2048] parity-folded layout for the output-side x add)."""
    xf = np.ascontiguousarray(np.asarray(x, np.float32)).reshape(TOK_TOTAL, D)
    shards = []
    for c in range(N_CORES):
        xc = xf[c * TOK_CORE : (c + 1) * TOK_CORE]          # [16384, 6]
        lane = xc.reshape(GROUPS, TPC, D).transpose(1, 2, 0).reshape(KP, GROUPS)
        xgc = np.empty((XROWS, GROUPS), np.float32)
        xgc[:KP] = lane
        xgc[KP] = 1.0
        # fold: row 32p+6tl+d, col 512s+cc  <-  lane col 512*(2s+p)+cc
        xbc = np.zeros((ORROWS, OGCOLS), np.float32)
        lf = lane.reshape(KP, N_TILES // 2, 2, FD)           # [24, s, p, cc]
        xbc[0:KP] = lf[:, :, 0, :].reshape(KP, OGCOLS)
        xbc[32 : 32 + KP] = lf[:, :, 1, :].reshape(KP, OGCOLS)
        shards.append((xgc, xbc))
    return shards


def _unshard_out(outs):
    """list of per-core [48, 2048] folded outputs -> [B,S,N,D]."""
    full = np.empty((TOK_TOTAL, D), np.float32)
    for c, ogc in enumerate(outs):
        ogc = np.asarray(ogc)
        # row 24p + 6tl + d, col 512s + cc -> token 4*(512*(2s+p)+cc)+tl
        o = np.asarray(ogc).reshape(2, KP, OGCOLS)
        o = o.reshape(2, TPC, D, N_TILES // 2, FD)          # [p, tl, d, s, cc]
        o = o.transpose(3, 0, 4, 1, 2).reshape(TOK_CORE, D)  # [s, p, cc, tl, d]
        full[c * TOK_CORE : (c + 1) * TOK_CORE] = o
    return full.reshape(B, S, N, D)


# Test-harness knobs (ignored in normal use): set kernel._TRACE = True to
# collect an NTFF profile; the BassKernelResults lands in kernel._LAST_RES.
_TRACE = False
_LAST_RES = None


def kernel(x, W_in, b_in, W1, b1, W2, b2, W3, b3):
    global _LAST_RES
    from concourse.bass_utils import run_bass_kernel_spmd

    nc = _get_nc()
    consts = _prep_weights(W_in, b_in, W1, b1, W2, b2, W3, b3)
    shards = _shard_x(x)
    in_maps = [
        {"xg": shards[c][0], "xb": shards[c][1], **consts} for c in range(N_CORES)
    ]
    res = run_bass_kernel_spmd(nc, in_maps, list(range(N_CORES)), trace=_TRACE)
    _LAST_RES = res
    return _unshard_out([res.results[c]["og"] for c in range(N_CORES)])


# revision 8
# speedup vs baseline: 1.8599x; 1.1429x over previous
"""Trainium2 Bass kernel for nn_HamiltonianVersorNN.

Math: the reference energy reads only blade-0 of the final layer, and the
versor gate h*sigmoid(h[...,0:1]) makes blade-0 evolve as elementwise SiLU.
Backprop therefore collapses exactly to a 2-layer SiLU MLP on blade-0:

    z1 = A x + c1            A  = W1 @ W_in[:, ::32].T          [32, 6]
    z2 = W2 silu(z1) + c2    c1 = W1 @ b_in[::32] + b1[:, 0]
    dx = A.T (W2.T (w3 * silu'(z2)) * silu'(z1))
    out = x + dt * [dx[3:6], -dx[0:3]]

Performance structure (vs the fp32 block-diag baseline):
  * float32r matmuls (FP22 multiply, 1 cycle/row at N=512, vs fp32's 4).
  * silu(z1) never materialized: with u1 = z1*tanh(z1/2),
    silu(z1) = (z1 + u1)/2, so W2 silu(z1) = 0.5*(W2 A)x + 0.5*W2 u1 +
    0.5*W2 c1 folds into two stationaries. ACT does only tanh + 2x
    silu' per tile; DVE does 2 muls.
  * biases ride a host-provided ones-row on the x tile (row 24), so no
    rank-1 bias matmuls and no ACT bias chains.
  * the last matmul runs bf16 so its PSUM output can col-tile: chunk
    parity p lands at partitions 32p+{0..23} of one [56, 512] bank,
    halving PSUM readout cost; the x-passthrough is added during the
    readout against a second, fold-layout copy of x (xb).

Sharding: pure data parallel over B*S*N positions, 8 cores, 16384
positions/core; partition 32*tl + c holds channel c of token 4g+tl.
(fp22 numerics verified on CPU: rel err 4.7e-5 vs jax reference.)
"""

import sys

import numpy as np

if "/opt/trn_rl_repo" not in sys.path:
    sys.path.insert(0, "/opt/trn_rl_repo")

import concourse.bass as bass
import concourse.tile as tile
from concourse import mybir

AF = mybir.ActivationFunctionType
F32 = mybir.dt.float32
F32R = mybir.dt.float32r

N_CORES = 8
B, S, N, D = 32, 256, 16, 6
HIDDEN = 32
BLADES = 32
DT = 0.01

TOK_TOTAL = B * S * N            # 131072 positions
TOK_CORE = TOK_TOTAL // N_CORES  # 16384
TPC = 4                          # tokens packed per 128-partition column
GROUPS = TOK_CORE // TPC         # 4096 columns per core
FD = 512                         # free-dim per tile (1 PSUM bank fp32)
N_TILES = GROUPS // FD           # 8

KP = TPC * D                     # 24 x/out partitions per lane block
XROWS = KP + 1                   # + ones row for bias folding
ORROWS = 56                      # folded out rows: 32p + (0..23), p in {0,1}
SFD = 2 * FD                     # 1024-column super-tile (ACT/DVE op span)
BF16 = mybir.dt.bfloat16
OGCOLS = GROUPS // 2             # 2048 output columns (2 chunks/bank)


def _build_nc():
    nc = bass.Bass()

    xg = nc.dram_tensor("xg", [XROWS, GROUPS], F32R, kind="ExternalInput")
    l1 = nc.dram_tensor("l1", [XROWS, 128], F32R, kind="ExternalInput")
    l2a = nc.dram_tensor("l2a", [XROWS, 128], F32R, kind="ExternalInput")
    l2b = nc.dram_tensor("l2b", [128, 128], F32R, kind="ExternalInput")
    l3 = nc.dram_tensor("l3", [128, 128], F32R, kind="ExternalInput")
    l4 = nc.dram_tensor("l4", [128, KP], BF16, kind="ExternalInput")
    xb = nc.dram_tensor("xb", [ORROWS, OGCOLS], F32, kind="ExternalInput")
    og = nc.dram_tensor("og", [2 * KP, OGCOLS], F32, kind="ExternalOutput")

    NSUP = N_TILES // 2

    with tile.TileContext(nc) as tc:
        with (
            tc.tile_pool(name="consts", bufs=1) as consts,
            tc.tile_pool(name="xin", bufs=1) as xin,
            tc.tile_pool(name="work", bufs=2) as work,
            tc.tile_pool(name="outp", bufs=2) as outp,
            tc.tile_pool(name="ps1", bufs=1, space="PSUM") as ps1,
            tc.tile_pool(name="ps2", bufs=2, space="PSUM") as ps2,
            tc.tile_pool(name="psv", bufs=1, space="PSUM") as psv,
            tc.tile_pool(name="pso", bufs=1, space="PSUM") as pso,
        ):
            # First x group first so mm1 of super 0 can start ASAP; the
            # remaining loads trickle in behind it on the same queue.
            xw = [xin.tile([XROWS, SFD], F32R, name=f"xw{g}") for g in range(NSUP)]
            nc.sync.dma_start(out=xw[0][:], in_=xg[:, bass.ts(0, SFD)])
            sb_l1 = consts.tile([XROWS, 128], F32R)
            nc.sync.dma_start(out=sb_l1[:], in_=l1[:])
            sb_l2a = consts.tile([XROWS, 128], F32R)
            nc.sync.dma_start(out=sb_l2a[:], in_=l2a[:])
            sb_l2b = consts.tile([128, 128], F32R)
            nc.sync.dma_start(out=sb_l2b[:], in_=l2b[:])
            sb_l3 = consts.tile([128, 128], F32R)
            nc.sync.dma_start(out=sb_l3[:], in_=l3[:])
            sb_l4 = consts.tile([128, KP], BF16)
            nc.sync.dma_start(out=sb_l4[:], in_=l4[:])
            for g in range(1, NSUP):
                nc.sync.dma_start(out=xw[g][:], in_=xg[:, bass.ts(g, SFD)])
            sb_xb = consts.tile([ORROWS, OGCOLS], F32)

            # Dummy first activation: walrus attaches the ACT table load to
            # the first Activation instruction, which can then carry only a
            # single sync wait. Give it a single-wait warm-up op.
            warm_in = consts.tile([1, 128], F32)
            nc.vector.memset(warm_in[:], 0.25)
            warm = consts.tile([1, 128], F32)
            nc.scalar.activation(warm[:], warm_in[:], AF.Derivative_silu)

            # Software pipeline with a one-super skew: FRONT(s) runs
            # mm1 -> tanh/silu' -> u1 -> mm2 for super s, BACK(s-1) finishes
            # the previous super (d2 -> mm3 -> g1 -> mm4 -> out). Emission
            # order = engine queue order, so each engine sees next-super
            # front work before it can stall on this super's back chain.
            front = {}

            def emit_front(s):
                xsl = [xw[s][:, bass.ts(p, FD)] for p in range(2)]
                # fold-layout x for the output add of BACK(s)
                nc.sync.dma_start(
                    out=sb_xb[:, bass.ts(s, FD)], in_=xb[:, bass.ts(s, FD)]
                )

                # z1 = blockdiag(A) @ x + c1 (bias via ones row)
                z1 = ps1.tile([128, SFD], F32, tag="z1", name=f"z1_{s}")
                for p in range(2):
                    nc.tensor.matmul(
                        z1[:, bass.ts(p, FD)], sb_l1[:], xsl[p], start=True, stop=True
                    )

                # t1 = tanh(z1/2), d1 = silu'(z1)
                t1 = work.tile([128, SFD], F32, tag="t1", name=f"t1_{s}")
                nc.scalar.activation(t1[:], z1[:], AF.Tanh, scale=0.5)
                d1 = work.tile([128, SFD], F32, tag="d1", name=f"d1_{s}")
                nc.scalar.activation(d1[:], z1[:], AF.Derivative_silu)

                # u1 = z1 * t1   (silu(z1) = (z1 + u1)/2, kept implicit)
                u1 = work.tile([128, SFD], F32R, tag="u1", name=f"u1_{s}")
                nc.vector.tensor_mul(u1[:], z1[:], t1[:])

                # z2 = 0.5*W2A x + (c2 + 0.5*W2 c1) + 0.5*W2 u1
                z2 = ps2.tile([128, SFD], F32, tag="z2", name=f"z2_{s}")
                for p in range(2):
                    zs = z2[:, bass.ts(p, FD)]
                    nc.tensor.matmul(zs, sb_l2a[:], xsl[p], start=True, stop=False)
                    nc.tensor.matmul(
                        zs, sb_l2b[:], u1[:, bass.ts(p, FD)], start=False, stop=True
                    )
                front[s] = (xsl, z2, d1)

            def emit_back(s):
                xsl, z2, d1 = front.pop(s)

                # d2 = silu'(z2)
                d2 = work.tile([128, SFD], F32R, tag="d2", name=f"d2_{s}")
                nc.scalar.activation(d2[:], z2[:], AF.Derivative_silu)

                # v1 = blockdiag(w3*W2)^T-contraction @ d2; g1 = v1 * d1
                # (bf16 g1 feeds the col-tiled bf16 mm4)
                po = pso.tile([ORROWS, FD], F32, tag="po", name=f"po_{s}")
                for p in range(2):
                    v1 = psv.tile([128, FD], F32, tag="v1", name=f"v1_{s}{p}")
                    nc.tensor.matmul(
                        v1[:], sb_l3[:], d2[:, bass.ts(p, FD)], start=True, stop=True
                    )
                    g1 = work.tile([128, FD], BF16, tag=f"g1{p}", name=f"g1_{s}{p}")
                    nc.vector.tensor_mul(g1[:], v1[:], d1[:, bass.ts(p, FD)])
                    # po[32p + (0..23)] = dt*J*(A^T g1), parity-folded
                    nc.tensor.matmul(
                        po[32 * p : 32 * p + KP, :],
                        sb_l4[:],
                        g1[:],
                        start=True,
                        stop=True,
                    )

                # out = x + po, added during PSUM readout (fold-layout x)
                osb = outp.tile([ORROWS, FD], F32, tag="osb", name=f"osb_{s}")
                nc.vector.tensor_add(osb[:], po[:], sb_xb[:, bass.ts(s, FD)])
                cs = bass.ts(s, FD)
                nc.gpsimd.dma_start(out=og[0:KP, cs], in_=osb[0:KP, :])
                nc.gpsimd.dma_start(out=og[KP : 2 * KP, cs], in_=osb[32 : 32 + KP, :])

            for s in range(NSUP):
                emit_front(s)
                if s > 0:
                    emit_back(s - 1)
            emit_back(NSUP - 1)

    return nc


def _split_multi_waits(nc):
    """This walrus build rejects engine instructions carrying more than one
    sync wait ("Too many sync wait commands"). Hoist all but one wait of
    each instruction onto standalone NoOps issued just before it on the
    same engine (engines execute their queue in order, so semantics are
    preserved)."""
    for f in nc.m.functions:
        for b in f.blocks:
            insts = list(b.instructions)
            out = []
            changed = False
            for inst in insts:
                # This walrus build also rejects the raw-ISA
                # EVENT_SEMAPHORE_RANGE_CLEAR Tile emits at context end
                # ("ISA wrong length" — ISA table version skew). The NEFF
                # preamble re-initializes semaphores, so drop it.
                if (
                    type(inst).__name__ == "InstISA"
                    and getattr(inst, "op_name", "") == "EVENT_SEMAPHORE_RANGE_CLEAR"
                ):
                    changed = True
                    continue
                si = getattr(inst, "sync_info", None)
                waits = list(si.on_wait) if si is not None and si.on_wait else []
                if len(waits) > 1:
                    changed = True
                    for k, w in enumerate(waits[:-1]):
                        nop = mybir.InstNoOp(name=f"{inst.name}-w{k}", ins=[], outs=[])
                        nop.engine = inst.engine
                        nop.sync_info = mybir.SyncInfo(on_wait=[w], on_update=[])
                        out.append(nop)
                    inst.sync_info = mybir.SyncInfo(
                        on_wait=[waits[-1]], on_update=list(si.on_update or [])
                    )
                out.append(inst)
            if changed:
                b.instructions = out
    return nc


_NC_CACHE = None


def _get_nc():
    global _NC_CACHE
    if _NC_CACHE is None:
        _NC_CACHE = _split_multi_waits(_build_nc())
    return _NC_CACHE


def _prep_weights(W_in, b_in, W1, b1, W2, b2, W3, b3):
    """Host-side constant folding into the kernel's stationary layouts."""
    W_in = np.asarray(W_in, np.float64)
    b_in = np.asarray(b_in, np.float64)
    W1 = np.asarray(W1, np.float64)
    b1 = np.asarray(b1, np.float64)
    W2 = np.asarray(W2, np.float64)
    b2 = np.asarray(b2, np.float64)
    W3 = np.asarray(W3, np.float64)

    Win0 = W_in[:, ::BLADES]            # [6, 8]
    bin0 = b_in[::BLADES]               # [8]
    A = W1 @ Win0.T                     # [32, 6]
    c1 = W1 @ bin0 + b1[:, 0]           # [32]
    c2 = b2[:, 0]                       # [32]
    w3 = W3[0, :]                       # [32]

    W2A = 0.5 * (W2 @ A)                # [32, 6]
    cz2 = 0.5 * (W2 @ c1) + c2          # [32]

    # Bout[d, c]: out[d] += dt*dx[d+3] (d<3), -dt*dx[d-3] (d>=3); dx = A^T g1
    Bout = np.zeros((D, HIDDEN))
    Bout[0:3, :] = DT * A[:, 3:6].T
    Bout[3:6, :] = -DT * A[:, 0:3].T

    import ml_dtypes

    l1 = np.zeros((XROWS, 128), np.float32)
    l2a = np.zeros((XROWS, 128), np.float32)
    l2b = np.zeros((128, 128), np.float32)
    l3 = np.zeros((128, 128), np.float32)
    l4 = np.zeros((128, KP), ml_dtypes.bfloat16)
    for tl in range(TPC):
        r0, c0 = 6 * tl, 32 * tl
        # z1[32tl+c] = sum_d A[c,d] x[d] + c1[c]
        l1[r0 : r0 + 6, c0 : c0 + 32] = A.T.astype(np.float32)
        l1[KP, c0 : c0 + 32] = c1.astype(np.float32)
        # z2 partial from x: 0.5*(W2A)[j,d] + bias row
        l2a[r0 : r0 + 6, c0 : c0 + 32] = W2A.T.astype(np.float32)
        l2a[KP, c0 : c0 + 32] = cz2.astype(np.float32)
        # z2 partial from u1: 0.5*W2[j,i]
        l2b[c0 : c0 + 32, c0 : c0 + 32] = (0.5 * W2.T).astype(np.float32)
        # v1[j] = sum_i w3[i] W2[i,j] d2[i]
        l3[c0 : c0 + 32, c0 : c0 + 32] = (w3[:, None] * W2).astype(np.float32)
        # out[6tl+d] += Bout[d, c] g1[32tl+c]
        l4[c0 : c0 + 32, r0 : r0 + 6] = Bout.T.astype(ml_dtypes.bfloat16)

    return {"l1": l1, "l2a": l2a, "l2b": l2b, "l3": l3, "l4": l4}


def _shard_x(x):
    """[B,S,N,D] -> per-core ([25, GROUPS] lane layout + ones row,
    [56, 2048] parity-folded layout for the output-side x add)."""
    xf = np.ascontiguousarray(np.asarray(x, np.float32)).reshape(TOK_TOTAL, D)
    shards = []
    for c in range(N_CORES):
        xc = xf[c * TOK_CORE : (c + 1) * TOK_CORE]          # [16384, 6]
        lane = xc.reshape(GROUPS, TPC, D).transpose(1, 2, 0).reshape(KP, GROUPS)
        xgc = np.empty((XROWS, GROUPS), np.float32)
        xgc[:KP] = lane
        xgc[KP] = 1.0
        # fold: row 32p+6tl+d, col 512s+cc  <-  lane col 512*(2s+p)+cc
        xbc = np.zeros((ORROWS, OGCOLS), np.float32)
        lf = lane.reshape(KP, N_TILES // 2, 2, FD)           # [24, s, p, cc]
        xbc[0:KP] = lf[:, :, 0, :].reshape(KP, OGCOLS)
        xbc[32 : 32 + KP] = lf[:, :, 1, :].reshape(KP, OGCOLS)
        shards.append((xgc, xbc))
    return shards


def _unshard_out(outs):
    """list of per-core [48, 2048] folded outputs -> [B,S,N,D]."""
    full = np.empty((TOK_TOTAL, D), np.float32)
    for c, ogc in enumerate(outs):
        ogc = np.asarray(ogc)
        # row 24p + 6tl + d, col 512s + cc -> token 4*(512*(2s+p)+cc)+tl
        o = np.asarray(ogc).reshape(2, KP, OGCOLS)
        o = o.reshape(2, TPC, D, N_TILES // 2, FD)          # [p, tl, d, s, cc]
        o = o.transpose(3, 0, 4, 1, 2).reshape(TOK_CORE, D)  # [s, p, cc, tl, d]
        full[c * TOK_CORE : (c + 1) * TOK_CORE] = o
    return full.reshape(B, S, N, D)


# Test-harness knobs (ignored in normal use): set kernel._TRACE = True to
# collect an NTFF profile; the BassKernelResults lands in kernel._LAST_RES.
_TRACE = False
_LAST_RES = None


def kernel(x, W_in, b_in, W1, b1, W2, b2, W3, b3):
    global _LAST_RES
    from concourse.bass_utils import run_bass_kernel_spmd

    nc = _get_nc()
    consts = _prep_weights(W_in, b_in, W1, b1, W2, b2, W3, b3)
    shards = _shard_x(x)
    in_maps = [
        {"xg": shards[c][0], "xb": shards[c][1], **consts} for c in range(N_CORES)
    ]
    res = run_bass_kernel_spmd(nc, in_maps, list(range(N_CORES)), trace=_TRACE)
    _LAST_RES = res
    return _unshard_out([res.results[c]["og"] for c in range(N_CORES)])
